# revision 40
# baseline (speedup 1.0000x reference)
"""DualContextAttention Trainium2 kernel.

Sharding: 8 cores = 4 batches x 2 query-halves. Each core (b, s) runs
attention for batch b over query positions n in [2048*s, 2048*(s+1)),
plus the pointwise tail (d1 gate, l1) for those positions (phase A).
Phase B consumes the gathered yl1 halo rows + global pooled sums and
runs the 3x3 conv stack, SE branch and final gating.

All BN layers are folded into the adjacent conv weights on the host
(inference-mode BN with fixed running stats => per-channel affine).
Softmax is computed without max-subtraction: |energy| < ~60 here, far
from fp32 exp overflow (88), and the ratio exp(e)/sum(exp(e)) is
unchanged. The softmax denominator is a cross-partition sum done on the
otherwise-idle GPSIMD engine; sigmoid is computed as 0.5*tanh(x/2)+0.5
in phase A so the ACT engine stays on the exp_and_others table set.
"""

import os
import numpy as np

import concourse.bass as bass
import concourse.tile as tile
from concourse import mybir
from concourse.alu_op_type import AluOpType
from concourse.bass_utils import run_bass_kernel_spmd
from bass_rust import AxisListType

F32 = mybir.dt.float32
MM_DT = mybir.dt.float32r if os.environ.get("KERNEL_MM_DT", "f32r") == "f32r" else F32
ACT = mybir.ActivationFunctionType

B, C, C2, H, W = 4, 256, 128, 64, 64
N = H * W          # 4096
NH = N // 2        # 2048 query positions per core
NG = NH // 512     # 512-wide query groups per core
EPS = 1e-5
NCORES = 8


def _split_multi_waits(nc, max_waits=1):
    """walrus in this container rejects instructions carrying more than one
    sync-wait; hoist extras onto preceding same-engine NoOps."""
    ctr = 0
    for f in nc.m.functions:
        for bb in f.blocks:
            insts = bb.instructions
            out = []
            changed = False
            for inst in insts:
                si = inst.sync_info
                if (
                    si is not None
                    and si.on_wait is not None
                    and len(si.on_wait) > max_waits
                ):
                    waits = list(si.on_wait)
                    for w in waits[:-max_waits]:
                        out.append(
                            mybir.InstNoOp(
                                name=f"wsplit-{ctr}",
                                engine=inst.engine,
                                sync_info=mybir.SyncInfo(on_wait=[w], on_update=[]),
                            )
                        )
                        ctr += 1
                    inst.sync_info = mybir.SyncInfo(
                        on_wait=waits[-max_waits:], on_update=list(si.on_update)
                    )
                    changed = True
                out.append(inst)
            if changed:
                bb.instructions = out
    return ctr


def _mm(nc, out, lhsT, rhs, start, stop):
    nc.tensor.matmul(out, lhsT, rhs, start=start, stop=stop)


# ---------------------------------------------------------------- phase A


def _build_phase_a():
    nc = bass.Bass()
    xb = nc.dram_tensor("xb", [2, 128, N], MM_DT, kind="ExternalInput")
    # all [128, k] weights packed into one DMA; [1, k] bias rows in another
    WPA = 256 + 256 + 516 + 512 + 512 + 128 + 2 + 2
    wpk = nc.dram_tensor("wpk", [128, WPA], MM_DT, kind="ExternalInput")
    wrow = nc.dram_tensor("wrow", [1, 514], MM_DT, kind="ExternalInput")

    feat_d = nc.dram_tensor("feat", [2, 128, NH], MM_DT, kind="ExternalOutput")
    yl1_d = nc.dram_tensor("yl1", [2, 128, NH], MM_DT, kind="ExternalOutput")
    ysum_d = nc.dram_tensor("ysum", [128, 2], F32, kind="ExternalOutput")

    with tile.TileContext(nc) as tc:
        with (
            tc.tile_pool(name="wp", bufs=1) as wp,
            tc.tile_pool(name="kqv", bufs=1) as kqv,
            tc.tile_pool(name="outp", bufs=1) as outp,
            tc.tile_pool(name="ps", bufs=1, space="PSUM") as ps,
        ):
            # ---- load weights (single packed DMA + one bias-row DMA)
            wpk_sb = wp.tile([128, WPA], MM_DT)
            nc.sync.dma_start(out=wpk_sb, in_=wpk[:, :])
            wrow_sb = wp.tile([1, 514], MM_DT)
            nc.sync.dma_start(out=wrow_sb, in_=wrow[:, :])
            o = 0
            wq_sb = wpk_sb[:, o : o + 256].rearrange("p (a m) -> p a m", a=2)
            o += 256
            wk_sb = wpk_sb[:, o : o + 256].rearrange("p (a m) -> p a m", a=2)
            o += 256
            wv_sb = wpk_sb[:, o : o + 516].rearrange("p (a m) -> p a m", a=2)
            o += 516
            d1w_sb = wpk_sb[:, o : o + 512].rearrange("p (a m) -> p a m", a=2)
            o += 512
            l1w_sb = wpk_sb[:, o : o + 512].rearrange("p (a m) -> p a m", a=2)
            o += 512
            ident_sb = wpk_sb[:, o : o + 128]
            o += 128
            d1b_sb = wpk_sb[:, o : o + 2].bitcast(F32)
            o += 2
            l1b_sb = wpk_sb[:, o : o + 2].bitcast(F32)
            bq_row = wrow_sb[:, 0:128]
            bk_row = wrow_sb[:, 128:256]
            bv_row = wrow_sb[:, 256 : 256 + 258]
            ones_f = wp.tile([1, 512], F32)
            nc.vector.memset(ones_f, 1.0)
            ones_row = wp.tile([1, 512], MM_DT)
            nc.vector.tensor_copy(ones_row, ones_f)
            ones_col = wp.tile([1, 128], MM_DT)
            nc.vector.tensor_copy(ones_col, ones_f[:, 0:128])

            warm = wp.tile([1, 1], F32)
            nc.scalar.activation(warm, ones_f[0:1, 0:1], ACT.Exp)
            k_sb = kqv.tile([128, N], MM_DT)
            q_sb = kqv.tile([128, NH], MM_DT)
            vT_sb = kqv.tile([128, 32, C + 2], MM_DT)

            feat_sb = outp.tile([128, 2, NH], MM_DT)
            ys_parts = outp.tile([128, 2, NG], F32)
            ys_sb = outp.tile([128, 2], F32)

            # ---- load x + projections (x freed after this block)
            with tc.tile_pool(name="xp", bufs=1) as xp:
                NC4 = N // 4
                xb_t = [
                    xp.tile([128, 2, NC4], MM_DT, name=f"xbc{c4}")
                    for c4 in range(4)
                ]
                for c4 in range(4):
                    for a in range(2):
                        eng = nc.sync if a == 0 else nc.scalar
                        eng.dma_start(
                            out=xb_t[c4][:, a, :],
                            in_=xb[a][:, bass.ts(c4, NC4)],
                        )

                def xb_sl(a, lo, width):
                    c4 = lo // NC4
                    assert lo + width <= (c4 + 1) * NC4
                    return xb_t[c4][:, a, lo - c4 * NC4 : lo - c4 * NC4 + width]

                def _pscopy(i, out, in_):
                    if i % 2 == 0:
                        nc.scalar.activation(out, in_, ACT.Copy)
                    else:
                        nc.vector.tensor_copy(out, in_)

                slot_tags = [("pva", 1), ("pvb", 1), ("e", 2), ("e", 2)]
                slot_i = [0]

                def _ptile(shape):
                    tg, bf = slot_tags[slot_i[0] % 4]
                    slot_i[0] += 1
                    return ps.tile(shape, F32, tag=tg, bufs=bf,
                                   name=f"proj{slot_i[0]}")

                for j in range(N // 512):
                    pk = _ptile([128, 512])
                    _mm(nc, pk, wk_sb[:, 0, :], xb_sl(0, j * 512, 512),
                        True, False)
                    _mm(nc, pk, wk_sb[:, 1, :], xb_sl(1, j * 512, 512),
                        False, False)
                    _mm(nc, pk, bk_row, ones_row, False, True)
                    _pscopy(j, k_sb[:, bass.ts(j, 512)], pk)
                for j in range(NH // 512):
                    pq = _ptile([128, 512])
                    _mm(nc, pq, wq_sb[:, 0, :], xb_sl(0, j * 512, 512),
                        True, False)
                    _mm(nc, pq, wq_sb[:, 1, :], xb_sl(1, j * 512, 512),
                        False, False)
                    _mm(nc, pq, bq_row, ones_row, False, True)
                    nc.vector.tensor_copy(q_sb[:, bass.ts(j, 512)], pq)

                for mb in range(32):
                    pv = _ptile([128, C + 2])
                    _mm(nc, pv, xb_sl(0, mb * 128, 128), wv_sb[:, 0, :],
                        True, False)
                    _mm(nc, pv, xb_sl(1, mb * 128, 128), wv_sb[:, 1, :],
                        False, False)
                    _mm(nc, pv, ones_col, bv_row, False, True)
                    _pscopy(mb, vT_sb[:, mb, :], pv)
            attn_ctx = tc.tile_pool(name="attn", bufs=1)
            small_ctx = tc.tile_pool(name="small", bufs=2)
            tmp_ctx = tc.tile_pool(name="tmp", bufs=2)
            attn = attn_ctx.__enter__()
            small = small_ctx.__enter__()
            tmp = tmp_ctx.__enter__()

            # ---- per 512-wide query group: attention + gate + l1,
            # software-pipelined one group deep (normalize/tail of group g
            # runs while group g+1's energy/exp stream is in flight).
            state = {}

            EGRP = [3, 3, 3, 3, 3, 3, 3, 3, 3, 3, 2]  # 32 chunks

            def emit_energy(g):
                gsl = bass.ts(g, 512)
                parts = []
                mb0 = 0
                for gi, w in enumerate(EGRP):
                    pe3 = ps.tile([128, w, 512], F32, tag="e", bufs=2,
                                  name=f"pe{gi}_{g}")
                    at3 = attn.tile([128, w, 512], MM_DT, tag=f"at{gi}",
                                    name=f"at{gi}_{g}")
                    for i in range(w):
                        _mm(nc, pe3[:, i, :], k_sb[:, bass.ts(mb0 + i, 128)],
                            q_sb[:, gsl], True, True)
                    nc.scalar.activation(at3, pe3, ACT.Exp)
                    parts.append((mb0, at3))
                    mb0 += w
                state[g] = {"parts": parts}

            def at_chunk(st, mb):
                for mb0, at3 in st["parts"]:
                    if mb0 <= mb < mb0 + at3.shape[1]:
                        return at3[:, mb - mb0, :]
                raise AssertionError(mb)

            def emit_pv(g):
                # PV transposed: attn chunk is the stationary operand, vT
                # (augmented with a ones column) moves. Output column C is
                # the softmax denominator for the 128 queries of the block.
                st = state[g]
                pvp = [None] * 4
                for half in range(2):
                    for nb in (2 * half, 2 * half + 1):
                        pvp[nb] = ps.tile(
                            [128, C + 2], F32,
                            tag="pva" if nb % 2 == 0 else "pvb",
                            bufs=1, name=f"pv{nb}_{g}")
                    for mb in range(32):
                        for nb in (2 * half, 2 * half + 1):
                            lhsT = at_chunk(st, mb)[:, bass.ts(nb, 128)]
                            _mm(nc, pvp[nb], lhsT, vT_sb[:, mb, :],
                                mb == 0, mb == 31)
                st["pvp"] = pvp

            def emit_norm_tail(g):
                gsl = bass.ts(g, 512)
                st = state[g]
                for nb in range(4):
                    pvp = st["pvp"][nb]
                    rc = small.tile([128, 1], F32, tag="recip")
                    nc.vector.reciprocal(rc, pvp[:, C : C + 1])
                    ftT = small.tile([128, C], MM_DT, tag="ftT")
                    nc.vector.tensor_scalar_mul(ftT, pvp[:, 0:C], rc)
                    for cb in range(2):
                        trp = ps.tile([128, 128], MM_DT, tag="e", bufs=2,
                                      name=f"trp{nb}_{cb}_{g}")
                        nc.tensor.transpose(
                            trp, ftT[:, bass.ts(cb, 128)], ident_sb
                        )
                        nc.vector.tensor_copy(
                            feat_sb[:, cb,
                                    g * 512 + nb * 128 : g * 512 + (nb + 1) * 128],
                            trp,
                        )
                for a in range(2):
                    nc.sync.dma_start(
                        out=feat_d[a][:, gsl], in_=feat_sb[:, a, gsl]
                    )
                ytiles = []
                for cb in range(2):
                    pz = ps.tile([128, 512], F32, tag="e", bufs=2)
                    _mm(nc, pz, d1w_sb[:, 0, bass.ts(cb, 128)],
                        feat_sb[:, 0, gsl], True, False)
                    _mm(nc, pz, d1w_sb[:, 1, bass.ts(cb, 128)],
                        feat_sb[:, 1, gsl], False, True)
                    zt = tmp.tile([128, 512], F32, tag="z")
                    nc.vector.tensor_scalar(
                        zt, pz, d1b_sb[:, cb : cb + 1], 0.0,
                        AluOpType.add, AluOpType.max,
                    )
                    th = tmp.tile([128, 512], F32, tag="th")
                    nc.scalar.activation(th, zt, ACT.Tanh, scale=0.5)
                    gt = tmp.tile([128, 512], F32, tag="g")
                    nc.vector.tensor_scalar(
                        gt, th, 0.5, 0.5, AluOpType.mult, AluOpType.add
                    )
                    yt = tmp.tile([128, 512], MM_DT, tag=f"y{cb}")
                    nc.vector.tensor_tensor(
                        yt, gt, feat_sb[:, cb, gsl], AluOpType.mult
                    )
                    nc.vector.reduce_sum(
                        ys_parts[:, cb, g : g + 1], yt, axis=AxisListType.X
                    )
                    ytiles.append(yt)
                for cb in range(2):
                    pl = ps.tile([128, 512], F32, tag="e", bufs=2)
                    _mm(nc, pl, l1w_sb[:, 0, bass.ts(cb, 128)], ytiles[0],
                        True, False)
                    _mm(nc, pl, l1w_sb[:, 1, bass.ts(cb, 128)], ytiles[1],
                        False, True)
                    yl1t = tmp.tile([128, 512], MM_DT, tag="yl1")
                    nc.vector.tensor_scalar(
                        yl1t, pl, l1b_sb[:, cb : cb + 1], 0.0,
                        AluOpType.add, AluOpType.max,
                    )
                    nc.sync.dma_start(out=yl1_d[cb][:, gsl], in_=yl1t)
                del state[g]

            emit_energy(0)
            for g in range(NG):
                if g > 0:
                    emit_norm_tail(g - 1)
                if g + 1 < NG:
                    emit_energy(g + 1)
                emit_pv(g)
            emit_norm_tail(NG - 1)

            for cb in range(2):
                nc.vector.reduce_sum(
                    ys_sb[:, cb : cb + 1], ys_parts[:, cb, :], axis=AxisListType.X
                )
            nc.sync.dma_start(out=ysum_d[:, :], in_=ys_sb)

            tmp_ctx.__exit__(None, None, None)
            small_ctx.__exit__(None, None, None)
            attn_ctx.__exit__(None, None, None)

    _split_multi_waits(nc)
    return nc


# ---------------------------------------------------------------- phase B


def _build_phase_b():
    nc = bass.Bass()
    feat = nc.dram_tensor("feat", [2, 128, NH], MM_DT, kind="ExternalInput")
    yl1h = nc.dram_tensor("yl1h", [2, 128, 34 * 66], MM_DT, kind="ExternalInput")
    yss = nc.dram_tensor("yss", [128, 4], F32, kind="ExternalInput")
    l2w0 = nc.dram_tensor("l2w0", [128, 9 * 2 * 128], MM_DT, kind="ExternalInput")
    WPB = 9 * 2 * 128 + 512 + 256 + 256 + 7
    wpb = nc.dram_tensor("wpb", [128, WPB], MM_DT, kind="ExternalInput")
    out_d = nc.dram_tensor("out", [2, 128, NH], F32, kind="ExternalOutput")

    with tile.TileContext(nc) as tc:
        with (
            tc.tile_pool(name="wp", bufs=1) as wp,
            tc.tile_pool(name="act", bufs=1) as actp,
            tc.tile_pool(name="tmp", bufs=3) as tmp,
            tc.tile_pool(name="ps", bufs=2, space="PSUM") as ps,
        ):
            l2w0_sb = wp.tile([128, 9 * 2 * 128], MM_DT)
            wpb_sb = wp.tile([128, WPB], MM_DT)
            nc.sync.dma_start(out=l2w0_sb, in_=l2w0[:, :])
            o = 9 * 2 * 128
            l2w_cb = [
                l2w0_sb.rearrange("p (t c m) -> p t c m", t=9, c=2),
                wpb_sb[:, 0:o].rearrange("p (t c m) -> p t c m", t=9, c=2),
            ]
            l3w_sb = wpb_sb[:, o : o + 512].rearrange("p (a m) -> p a m", a=2)
            o += 512
            r1w_sb = wpb_sb[:, o : o + 256].bitcast(F32).rearrange(
                "p (a m) -> p a m", a=2
            )
            o += 256
            r2w_sb = wpb_sb[:, o : o + 256].bitcast(F32)
            o += 256
            l2b_sb = wpb_sb[:, o : o + 2].bitcast(F32)
            o += 2
            l3b_sb = wpb_sb[:, o : o + 2].bitcast(F32)
            o += 2
            r1b_sb = wpb_sb[:, o : o + 1].bitcast(F32)
            o += 1
            r2b_sb = wpb_sb[:, o : o + 2].bitcast(F32)

            warm_f = wp.tile([1, 1], F32)
            nc.vector.memset(warm_f, 0.0)
            warm = wp.tile([1, 1], F32)
            nc.scalar.activation(warm, warm_f, ACT.Sigmoid)
            feat_sb = actp.tile([128, 2, NH], MM_DT)
            pad_sb = actp.tile([128, 2, 34, 66], MM_DT)
            out_sb = actp.tile([128, 2, NH], F32)
            yss_sb = actp.tile([128, 4], F32)
            pooled = actp.tile([128, 2], F32)
            yr1_sb = actp.tile([128, 1], F32)
            yr_sb = actp.tile([128, 2], F32)

            for a in range(2):
                nc.sync.dma_start(
                    out=pad_sb[:, a, :, :],
                    in_=yl1h[a].rearrange("p (r c) -> p r c", c=66),
                )
            nc.sync.dma_start(out=wpb_sb, in_=wpb[:, :])
            nc.sync.dma_start(out=yss_sb, in_=yss[:, :])
            for j in range(4):
                sl = bass.ts(j, 512)
                nc.sync.dma_start(
                    out=feat_sb[:, :, sl],
                    in_=feat[:, :, sl].rearrange("b p n -> p b n"),
                )

            # ---- SE branch (tiny)
            nc.vector.tensor_tensor(
                pooled, yss_sb[:, 0:2], yss_sb[:, 2:4], AluOpType.add
            )
            nc.vector.tensor_scalar_mul(pooled, pooled, 1.0 / N)
            pr = ps.tile([128, 1], F32, tag="tiny")
            _mm(nc, pr, r1w_sb[:, 0, :], pooled[:, 0:1], True, False)
            _mm(nc, pr, r1w_sb[:, 1, :], pooled[:, 1:2], False, True)
            nc.vector.tensor_scalar(
                yr1_sb, pr, r1b_sb, 0.0, AluOpType.add, AluOpType.max
            )
            for cb in range(2):
                pr2 = ps.tile([128, 1], F32, tag="tiny")
                _mm(nc, pr2, r2w_sb[:, bass.ts(cb, 128)], yr1_sb, True, True)
                nc.vector.tensor_scalar_add(
                    yr_sb[:, cb : cb + 1], pr2, r2b_sb[:, cb : cb + 1]
                )

            # ---- l2 (3x3 conv as 9 shifted 1x1 convs) -> l3 -> gate,
            # j-major with per-chunk yl2 tiles so l3 pipelines behind l2
            for j in range(4):
                sl = bass.ts(j, 512)
                yl2j = []
                for cb in range(2):
                    pl2 = ps.tile([128, 8, 64], F32, tag="l2", bufs=3)
                    first = True
                    for ci in range(2):
                        for ty in range(3):
                            for tx in range(3):
                                t = ty * 3 + tx
                                _mm(
                                    nc, pl2,
                                    l2w_cb[cb][:, t, ci, :],
                                    pad_sb[:, ci, j * 8 + ty : j * 8 + ty + 8,
                                           tx : tx + 64],
                                    first, t == 8 and ci == 1,
                                )
                                first = False
                    y2t = tmp.tile([128, 512], MM_DT, tag=f"y2{cb}")
                    nc.vector.tensor_scalar(
                        y2t,
                        pl2.rearrange("p a b -> p (a b)"),
                        l2b_sb[:, cb : cb + 1], 0.0,
                        AluOpType.add, AluOpType.max,
                    )
                    yl2j.append(y2t)
                for cb in range(2):
                    pl3 = ps.tile([128, 512], F32, tag="l3", bufs=2)
                    _mm(nc, pl3, l3w_sb[:, 0, bass.ts(cb, 128)], yl2j[0],
                        True, False)
                    _mm(nc, pl3, l3w_sb[:, 1, bass.ts(cb, 128)], yl2j[1],
                        False, True)
                    y3 = tmp.tile([128, 512], F32, tag="y3")
                    nc.vector.tensor_scalar(
                        y3, pl3, l3b_sb[:, cb : cb + 1], 0.0,
                        AluOpType.add, AluOpType.max,
                    )
                    gt = tmp.tile([128, 512], F32, tag="g")
                    nc.scalar.activation(
                        gt, y3, ACT.Sigmoid, bias=yr_sb[:, cb : cb + 1]
                    )
                    nc.vector.tensor_tensor(
                        out_sb[:, cb, sl], gt, feat_sb[:, cb, sl], AluOpType.mult
                    )
                    nc.sync.dma_start(out=out_d[cb][:, sl], in_=out_sb[:, cb, sl])

    _split_multi_waits(nc)
    return nc


# ---------------------------------------------------------------- host side

_CACHE = {}


def _programs():
    if "a" not in _CACHE:
        _CACHE["a"] = _build_phase_a()
        _CACHE["b"] = _build_phase_b()
    return _CACHE["a"], _CACHE["b"]


def _fold_bn(w, bias, g, bb, m, v):
    s = g / np.sqrt(v + EPS)
    t = bb - s * m
    wf = w * (s[:, None] if w.ndim == 2 else s[:, None, None, None])
    return wf.astype(np.float32), (s * bias + t).astype(np.float32)


def _prep_weights(inp):
    def fold(wk, bk2, pre):
        return _fold_bn(inp[wk], inp[bk2], inp[pre + "_g"], inp[pre + "_b"],
                        inp[pre + "_m"], inp[pre + "_v"])

    d1w, d1b = fold("d1_w", "d1_b", "bn1")
    l1w, l1b = fold("l1_w", "l1_b", "lbn1")
    l2w, l2b = fold("l2_w", "l2_b", "lbn2")
    l3w, l3b = fold("l3_w", "l3_b", "lbn3")
    r1w, r1b = fold("r1_w", "r1_b", "rbn")

    def pm(w):  # [ci, m] -> [128, 2*m] (per-partition packed, chunk-major)
        return np.ascontiguousarray(
            w.reshape(2, 128, -1).transpose(1, 0, 2).reshape(128, -1)
        )

    wv_aug = np.concatenate([inp["wv"].T, np.zeros((C, 2), np.float32)], axis=1)
    f32v = lambda a: np.ascontiguousarray(a).astype(np.float32)
    p = {}
    p["wpk"] = np.concatenate(
        [
            pm(inp["wq"].T), pm(inp["wk"].T), pm(wv_aug), pm(d1w.T), pm(l1w.T),
            np.eye(128, dtype=np.float32),
            f32v(d1b.reshape(2, 128).T), f32v(l1b.reshape(2, 128).T),
        ],
        axis=1,
    )
    p["wrow"] = np.concatenate(
        [inp["bq"], inp["bk"],
         np.concatenate([inp["bv"], [1.0, 0.0]]).astype(np.float32)]
    )[None, :]
    # phase B packs
    w2 = l2w.reshape(2, 128, 2, 128, 3, 3)  # [cb, co, ci_hi, ci_lo, ty, tx]
    l2p = np.ascontiguousarray(w2.transpose(3, 4, 5, 2, 0, 1))  # [ci_lo,ty,tx,ci_hi,cb,co]
    l2cb = [
        np.ascontiguousarray(l2p[:, :, :, :, cb, :]).reshape(128, -1)
        for cb in range(2)
    ]
    p["l2w0"] = l2cb[0]
    p["wpb"] = np.concatenate(
        [
            l2cb[1], pm(l3w.T), pm(r1w.T),
            f32v(inp["r2_w"].T),
            f32v(l2b.reshape(2, 128).T), f32v(l3b.reshape(2, 128).T),
            f32v(r1b[:, None]), f32v(inp["r2_b"].reshape(2, 128).T),
        ],
        axis=1,
    )
    return {k: np.ascontiguousarray(v.astype(np.float32)) for k, v in p.items()}


def _run_spmd(nc, in_maps):
    try:
        return run_bass_kernel_spmd(nc, in_maps, core_ids=list(range(NCORES)))
    except Exception:
        # transient NRT device errors have been observed; retry once
        return run_bass_kernel_spmd(nc, in_maps, core_ids=list(range(NCORES)))


def kernel(**inputs):
    nca, ncb = _programs()
    p = _prep_weights(inputs)
    x = inputs["x"].astype(np.float32).reshape(B, 2, 128, N)

    in_maps_a = []
    for core in range(NCORES):
        b, s = divmod(core, 2)
        m = {"wpk": p["wpk"], "wrow": p["wrow"]}
        if s == 0:
            m["xb"] = x[b]
        else:
            # rotate so this core's query half occupies columns [0, NH);
            # attention is permutation-invariant over key positions, so K/V
            # built from the rotated order give identical results.
            m["xb"] = np.ascontiguousarray(
                np.concatenate([x[b][:, :, NH:], x[b][:, :, :NH]], axis=2)
            )
        in_maps_a.append(m)
    res_a = _run_spmd(nca, in_maps_a)
    feats = [r["feat"] for r in res_a.results]
    yl1s = [r["yl1"].reshape(2, 128, 32, 64) for r in res_a.results]
    ysums = [r["ysum"] for r in res_a.results]

    zrow = np.zeros((2, 128, 1, 64), np.float32)
    in_maps_b = []
    for core in range(NCORES):
        b, s = divmod(core, 2)
        m = {"l2w0": p["l2w0"], "wpb": p["wpb"]}
        m["feat"] = feats[core]
        own, other = yl1s[core], yl1s[2 * b + (1 - s)]
        if s == 0:
            ylh = np.concatenate([zrow, own, other[:, :, 0:1]], axis=2)
        else:
            ylh = np.concatenate([other[:, :, 31:32], own, zrow], axis=2)
        ylh = np.pad(ylh, ((0, 0), (0, 0), (0, 0), (1, 1)))
        m["yl1h"] = np.ascontiguousarray(ylh).reshape(2, 128, 34 * 66)
        m["yss"] = np.concatenate([ysums[2 * b], ysums[2 * b + 1]], axis=1)
        in_maps_b.append(m)
    res_b = _run_spmd(ncb, in_maps_b)

    out = np.empty((B, C, H, W), np.float32)
    for core in range(NCORES):
        b, s = divmod(core, 2)
        out[b, :, s * 32 : (s + 1) * 32, :] = (
            res_b.results[core]["out"].reshape(C, 32, 64)
        )
    return out


# revision 41
# speedup vs baseline: 1.0774x; 1.0774x over previous
"""DualContextAttention Trainium2 kernel.

Sharding: 8 cores = 4 batches x 2 query-halves. Each core (b, s) runs
attention for batch b over query positions n in [2048*s, 2048*(s+1)),
plus the pointwise tail (d1 gate, l1) for those positions (phase A).
Phase B consumes the gathered yl1 halo rows + global pooled sums and
runs the 3x3 conv stack, SE branch and final gating.

All BN layers are folded into the adjacent conv weights on the host
(inference-mode BN with fixed running stats => per-channel affine).
Softmax is computed without max-subtraction: |energy| < ~60 here, far
from fp32 exp overflow (88), and the ratio exp(e)/sum(exp(e)) is
unchanged. The softmax denominator is a cross-partition sum done on the
otherwise-idle GPSIMD engine; sigmoid is computed as 0.5*tanh(x/2)+0.5
in phase A so the ACT engine stays on the exp_and_others table set.
"""

import os
import numpy as np

import concourse.bass as bass
import concourse.tile as tile
from concourse import mybir
from concourse.alu_op_type import AluOpType
from concourse.bass_utils import run_bass_kernel_spmd
from bass_rust import AxisListType

F32 = mybir.dt.float32
MM_DT = mybir.dt.float32r if os.environ.get("KERNEL_MM_DT", "f32r") == "f32r" else F32
ACT = mybir.ActivationFunctionType

B, C, C2, H, W = 4, 256, 128, 64, 64
N = H * W          # 4096
NH = N // 2        # 2048 query positions per core
NG = NH // 512     # 512-wide query groups per core
EPS = 1e-5
NCORES = 8


def _split_multi_waits(nc, max_waits=1):
    """walrus in this container rejects instructions carrying more than one
    sync-wait; hoist extras onto preceding same-engine NoOps."""
    ctr = 0
    for f in nc.m.functions:
        for bb in f.blocks:
            insts = bb.instructions
            out = []
            changed = False
            for inst in insts:
                si = inst.sync_info
                if (
                    si is not None
                    and si.on_wait is not None
                    and len(si.on_wait) > max_waits
                ):
                    waits = list(si.on_wait)
                    for w in waits[:-max_waits]:
                        out.append(
                            mybir.InstNoOp(
                                name=f"wsplit-{ctr}",
                                engine=inst.engine,
                                sync_info=mybir.SyncInfo(on_wait=[w], on_update=[]),
                            )
                        )
                        ctr += 1
                    inst.sync_info = mybir.SyncInfo(
                        on_wait=waits[-max_waits:], on_update=list(si.on_update)
                    )
                    changed = True
                out.append(inst)
            if changed:
                bb.instructions = out
    return ctr


def _mm(nc, out, lhsT, rhs, start, stop):
    nc.tensor.matmul(out, lhsT, rhs, start=start, stop=stop)


# ---------------------------------------------------------------- phase A


def _build_phase_a():
    nc = bass.Bass()
    xb = nc.dram_tensor("xb", [2, 128, N], MM_DT, kind="ExternalInput")
    # all [128, k] weights packed into one DMA; [1, k] bias rows in another
    WPA = 256 + 256 + 516 + 512 + 512 + 128 + 2 + 2
    wpk = nc.dram_tensor("wpk", [128, WPA], MM_DT, kind="ExternalInput")
    wrow = nc.dram_tensor("wrow", [1, 514], MM_DT, kind="ExternalInput")

    feat_d = nc.dram_tensor("feat", [2, 128, NH], MM_DT, kind="ExternalOutput")
    yl1_d = nc.dram_tensor("yl1", [2, 128, NH], MM_DT, kind="ExternalOutput")
    ysum_d = nc.dram_tensor("ysum", [128, 2], F32, kind="ExternalOutput")

    with tile.TileContext(nc) as tc:
        with (
            tc.tile_pool(name="wp", bufs=1) as wp,
            tc.tile_pool(name="kqv", bufs=1) as kqv,
            tc.tile_pool(name="outp", bufs=1) as outp,
            tc.tile_pool(name="ps", bufs=1, space="PSUM") as ps,
        ):
            # ---- load weights (single packed DMA + one bias-row DMA)
            wpk_sb = wp.tile([128, WPA], MM_DT)
            nc.sync.dma_start(out=wpk_sb, in_=wpk[:, :])
            wrow_sb = wp.tile([1, 514], MM_DT)
            nc.sync.dma_start(out=wrow_sb, in_=wrow[:, :])
            o = 0
            wq_sb = wpk_sb[:, o : o + 256].rearrange("p (a m) -> p a m", a=2)
            o += 256
            wk_sb = wpk_sb[:, o : o + 256].rearrange("p (a m) -> p a m", a=2)
            o += 256
            wv_sb = wpk_sb[:, o : o + 516].rearrange("p (a m) -> p a m", a=2)
            o += 516
            d1w_sb = wpk_sb[:, o : o + 512].rearrange("p (a m) -> p a m", a=2)
            o += 512
            l1w_sb = wpk_sb[:, o : o + 512].rearrange("p (a m) -> p a m", a=2)
            o += 512
            ident_sb = wpk_sb[:, o : o + 128]
            o += 128
            d1b_sb = wpk_sb[:, o : o + 2].bitcast(F32)
            o += 2
            l1b_sb = wpk_sb[:, o : o + 2].bitcast(F32)
            bq_row = wrow_sb[:, 0:128]
            bk_row = wrow_sb[:, 128:256]
            bv_row = wrow_sb[:, 256 : 256 + 258]
            ones_f = wp.tile([1, 512], F32)
            nc.vector.memset(ones_f, 1.0)
            ones_row = wp.tile([1, 512], MM_DT)
            nc.vector.tensor_copy(ones_row, ones_f)
            ones_col = wp.tile([1, 128], MM_DT)
            nc.vector.tensor_copy(ones_col, ones_f[:, 0:128])

            warm = wp.tile([1, 1], F32)
            nc.scalar.activation(warm, ones_f[0:1, 0:1], ACT.Exp)
            k_sb = kqv.tile([128, N], MM_DT)
            q_sb = kqv.tile([128, NH], MM_DT)
            vT_sb = kqv.tile([128, 32, C + 2], MM_DT)

            feat_sb = outp.tile([128, 2, NH], MM_DT)
            ys_parts = outp.tile([128, 2, NG], F32)
            ys_sb = outp.tile([128, 2], F32)

            # ---- load x + projections (x freed after this block)
            with tc.tile_pool(name="xp", bufs=1) as xp:
                NC4 = N // 4
                xb_t = [
                    xp.tile([128, 2, NC4], MM_DT, name=f"xbc{c4}")
                    for c4 in range(4)
                ]
                for c4 in range(4):
                    for a in range(2):
                        eng = nc.sync if a == 0 else nc.scalar
                        eng.dma_start(
                            out=xb_t[c4][:, a, :],
                            in_=xb[a][:, bass.ts(c4, NC4)],
                        )

                def xb_sl(a, lo, width):
                    c4 = lo // NC4
                    assert lo + width <= (c4 + 1) * NC4
                    return xb_t[c4][:, a, lo - c4 * NC4 : lo - c4 * NC4 + width]

                def _pscopy(i, out, in_):
                    if i % 2 == 0:
                        nc.scalar.activation(out, in_, ACT.Copy)
                    else:
                        nc.vector.tensor_copy(out, in_)

                slot_tags = [("pv0", 1), ("pv1", 1), ("pv2", 1), ("pv3", 1),
                             ("e", 2), ("e", 2)]
                slot_i = [0]

                def _ptile(shape):
                    tg, bf = slot_tags[slot_i[0] % 6]
                    slot_i[0] += 1
                    return ps.tile(shape, F32, tag=tg, bufs=bf,
                                   name=f"proj{slot_i[0]}")

                for j in range(N // 512):
                    pk = _ptile([128, 512])
                    _mm(nc, pk, wk_sb[:, 0, :], xb_sl(0, j * 512, 512),
                        True, False)
                    _mm(nc, pk, wk_sb[:, 1, :], xb_sl(1, j * 512, 512),
                        False, False)
                    _mm(nc, pk, bk_row, ones_row, False, True)
                    _pscopy(j, k_sb[:, bass.ts(j, 512)], pk)
                for j in range(NH // 512):
                    pq = _ptile([128, 512])
                    _mm(nc, pq, wq_sb[:, 0, :], xb_sl(0, j * 512, 512),
                        True, False)
                    _mm(nc, pq, wq_sb[:, 1, :], xb_sl(1, j * 512, 512),
                        False, False)
                    _mm(nc, pq, bq_row, ones_row, False, True)
                    nc.vector.tensor_copy(q_sb[:, bass.ts(j, 512)], pq)

                for mb in range(32):
                    pv = _ptile([128, C + 2])
                    _mm(nc, pv, xb_sl(0, mb * 128, 128), wv_sb[:, 0, :],
                        True, False)
                    _mm(nc, pv, xb_sl(1, mb * 128, 128), wv_sb[:, 1, :],
                        False, False)
                    _mm(nc, pv, ones_col, bv_row, False, True)
                    _pscopy(mb, vT_sb[:, mb, :], pv)
            attn_ctx = tc.tile_pool(name="attn", bufs=1)
            small_ctx = tc.tile_pool(name="small", bufs=2)
            tmp_ctx = tc.tile_pool(name="tmp", bufs=2)
            attn = attn_ctx.__enter__()
            small = small_ctx.__enter__()
            tmp = tmp_ctx.__enter__()

            # ---- per 512-wide query group: attention + gate + l1,
            # software-pipelined one group deep (normalize/tail of group g
            # runs while group g+1's energy/exp stream is in flight).
            state = {}

            def emit_energy(g):
                gsl = bass.ts(g, 512)
                parts = [
                    attn.tile([128, 2, 512], MM_DT, tag=f"at{mp}",
                              name=f"at{mp}_{g}")
                    for mp in range(16)
                ]
                for mp in range(16):
                    pe2 = ps.tile([128, 2, 512], F32, tag="e", bufs=2)
                    _mm(nc, pe2[:, 0, :], k_sb[:, bass.ts(2 * mp, 128)],
                        q_sb[:, gsl], True, True)
                    _mm(nc, pe2[:, 1, :], k_sb[:, bass.ts(2 * mp + 1, 128)],
                        q_sb[:, gsl], True, True)
                    nc.scalar.activation(parts[mp], pe2, ACT.Exp)
                state[g] = {"parts": parts}

            def at_chunk(st, mb):
                return st["parts"][mb // 2][:, mb % 2, :]

            def emit_pv(g):
                # PV transposed: attn chunk is the stationary operand, vT
                # (augmented with a ones column) moves. Output column C is
                # the softmax denominator for the 128 queries of the block.
                st = state[g]
                pvp = [
                    ps.tile([128, C + 2], F32, tag=f"pv{nb}", bufs=1,
                            name=f"pv{nb}_{g}")
                    for nb in range(4)
                ]
                for mb in range(32):
                    for nb in range(4):
                        lhsT = at_chunk(st, mb)[:, bass.ts(nb, 128)]
                        _mm(nc, pvp[nb], lhsT, vT_sb[:, mb, :],
                            mb == 0, mb == 31)
                st["pvp"] = pvp

            def emit_norm_tail(g):
                gsl = bass.ts(g, 512)
                st = state[g]
                for nb in range(4):
                    pvp = st["pvp"][nb]
                    rc = small.tile([128, 1], F32, tag="recip")
                    nc.vector.reciprocal(rc, pvp[:, C : C + 1])
                    ftT = small.tile([128, C], MM_DT, tag="ftT")
                    nc.vector.tensor_scalar_mul(ftT, pvp[:, 0:C], rc)
                    for cb in range(2):
                        trp = ps.tile([128, 128], MM_DT, tag=f"pv{nb}",
                                      bufs=1, name=f"trp{nb}_{cb}_{g}")
                        nc.tensor.transpose(
                            trp, ftT[:, bass.ts(cb, 128)], ident_sb
                        )
                        nc.vector.tensor_copy(
                            feat_sb[:, cb,
                                    g * 512 + nb * 128 : g * 512 + (nb + 1) * 128],
                            trp,
                        )
                for a in range(2):
                    nc.sync.dma_start(
                        out=feat_d[a][:, gsl], in_=feat_sb[:, a, gsl]
                    )
                ytiles = []
                for cb in range(2):
                    pz = ps.tile([128, 512], F32, tag="e", bufs=2)
                    _mm(nc, pz, d1w_sb[:, 0, bass.ts(cb, 128)],
                        feat_sb[:, 0, gsl], True, False)
                    _mm(nc, pz, d1w_sb[:, 1, bass.ts(cb, 128)],
                        feat_sb[:, 1, gsl], False, True)
                    zt = tmp.tile([128, 512], F32, tag="z")
                    nc.vector.tensor_scalar(
                        zt, pz, d1b_sb[:, cb : cb + 1], 0.0,
                        AluOpType.add, AluOpType.max,
                    )
                    th = tmp.tile([128, 512], F32, tag="th")
                    nc.scalar.activation(th, zt, ACT.Tanh, scale=0.5)
                    gt = tmp.tile([128, 512], F32, tag="g")
                    nc.vector.tensor_scalar(
                        gt, th, 0.5, 0.5, AluOpType.mult, AluOpType.add
                    )
                    yt = tmp.tile([128, 512], MM_DT, tag=f"y{cb}")
                    nc.vector.tensor_tensor(
                        yt, gt, feat_sb[:, cb, gsl], AluOpType.mult
                    )
                    nc.vector.reduce_sum(
                        ys_parts[:, cb, g : g + 1], yt, axis=AxisListType.X
                    )
                    ytiles.append(yt)
                for cb in range(2):
                    pl = ps.tile([128, 512], F32, tag="e", bufs=2)
                    _mm(nc, pl, l1w_sb[:, 0, bass.ts(cb, 128)], ytiles[0],
                        True, False)
                    _mm(nc, pl, l1w_sb[:, 1, bass.ts(cb, 128)], ytiles[1],
                        False, True)
                    yl1t = tmp.tile([128, 512], MM_DT, tag="yl1")
                    nc.vector.tensor_scalar(
                        yl1t, pl, l1b_sb[:, cb : cb + 1], 0.0,
                        AluOpType.add, AluOpType.max,
                    )
                    nc.sync.dma_start(out=yl1_d[cb][:, gsl], in_=yl1t)
                del state[g]

            emit_energy(0)
            for g in range(NG):
                if g > 0:
                    emit_norm_tail(g - 1)
                if g + 1 < NG:
                    emit_energy(g + 1)
                emit_pv(g)
            emit_norm_tail(NG - 1)

            for cb in range(2):
                nc.vector.reduce_sum(
                    ys_sb[:, cb : cb + 1], ys_parts[:, cb, :], axis=AxisListType.X
                )
            nc.sync.dma_start(out=ysum_d[:, :], in_=ys_sb)

            tmp_ctx.__exit__(None, None, None)
            small_ctx.__exit__(None, None, None)
            attn_ctx.__exit__(None, None, None)

    _split_multi_waits(nc)
    return nc


# ---------------------------------------------------------------- phase B


def _build_phase_b():
    nc = bass.Bass()
    feat = nc.dram_tensor("feat", [2, 128, NH], MM_DT, kind="ExternalInput")
    yl1h = nc.dram_tensor("yl1h", [2, 128, 34 * 66], MM_DT, kind="ExternalInput")
    yss = nc.dram_tensor("yss", [128, 4], F32, kind="ExternalInput")
    l2w0 = nc.dram_tensor("l2w0", [128, 9 * 2 * 128], MM_DT, kind="ExternalInput")
    WPB = 9 * 2 * 128 + 512 + 256 + 256 + 7
    wpb = nc.dram_tensor("wpb", [128, WPB], MM_DT, kind="ExternalInput")
    out_d = nc.dram_tensor("out", [2, 128, NH], F32, kind="ExternalOutput")

    with tile.TileContext(nc) as tc:
        with (
            tc.tile_pool(name="wp", bufs=1) as wp,
            tc.tile_pool(name="act", bufs=1) as actp,
            tc.tile_pool(name="tmp", bufs=3) as tmp,
            tc.tile_pool(name="ps", bufs=2, space="PSUM") as ps,
        ):
            l2w0_sb = wp.tile([128, 9 * 2 * 128], MM_DT)
            wpb_sb = wp.tile([128, WPB], MM_DT)
            nc.sync.dma_start(out=l2w0_sb, in_=l2w0[:, :])
            o = 9 * 2 * 128
            l2w_cb = [
                l2w0_sb.rearrange("p (t c m) -> p t c m", t=9, c=2),
                wpb_sb[:, 0:o].rearrange("p (t c m) -> p t c m", t=9, c=2),
            ]
            l3w_sb = wpb_sb[:, o : o + 512].rearrange("p (a m) -> p a m", a=2)
            o += 512
            r1w_sb = wpb_sb[:, o : o + 256].bitcast(F32).rearrange(
                "p (a m) -> p a m", a=2
            )
            o += 256
            r2w_sb = wpb_sb[:, o : o + 256].bitcast(F32)
            o += 256
            l2b_sb = wpb_sb[:, o : o + 2].bitcast(F32)
            o += 2
            l3b_sb = wpb_sb[:, o : o + 2].bitcast(F32)
            o += 2
            r1b_sb = wpb_sb[:, o : o + 1].bitcast(F32)
            o += 1
            r2b_sb = wpb_sb[:, o : o + 2].bitcast(F32)

            warm_f = wp.tile([1, 1], F32)
            nc.vector.memset(warm_f, 0.0)
            warm = wp.tile([1, 1], F32)
            nc.scalar.activation(warm, warm_f, ACT.Sigmoid)
            feat_sb = actp.tile([128, 2, NH], MM_DT)
            pad_sb = actp.tile([128, 2, 34, 66], MM_DT)
            out_sb = actp.tile([128, 2, NH], F32)
            yss_sb = actp.tile([128, 4], F32)
            pooled = actp.tile([128, 2], F32)
            yr1_sb = actp.tile([128, 1], F32)
            yr_sb = actp.tile([128, 2], F32)

            for a in range(2):
                nc.sync.dma_start(
                    out=pad_sb[:, a, :, :],
                    in_=yl1h[a].rearrange("p (r c) -> p r c", c=66),
                )
            nc.sync.dma_start(out=wpb_sb, in_=wpb[:, :])
            nc.sync.dma_start(out=yss_sb, in_=yss[:, :])
            for j in range(4):
                sl = bass.ts(j, 512)
                nc.sync.dma_start(
                    out=feat_sb[:, :, sl],
                    in_=feat[:, :, sl].rearrange("b p n -> p b n"),
                )

            # ---- SE branch (tiny)
            nc.vector.tensor_tensor(
                pooled, yss_sb[:, 0:2], yss_sb[:, 2:4], AluOpType.add
            )
            nc.vector.tensor_scalar_mul(pooled, pooled, 1.0 / N)
            pr = ps.tile([128, 1], F32, tag="tiny")
            _mm(nc, pr, r1w_sb[:, 0, :], pooled[:, 0:1], True, False)
            _mm(nc, pr, r1w_sb[:, 1, :], pooled[:, 1:2], False, True)
            nc.vector.tensor_scalar(
                yr1_sb, pr, r1b_sb, 0.0, AluOpType.add, AluOpType.max
            )
            for cb in range(2):
                pr2 = ps.tile([128, 1], F32, tag="tiny")
                _mm(nc, pr2, r2w_sb[:, bass.ts(cb, 128)], yr1_sb, True, True)
                nc.vector.tensor_scalar_add(
                    yr_sb[:, cb : cb + 1], pr2, r2b_sb[:, cb : cb + 1]
                )

            # ---- l2 (3x3 conv as 9 shifted 1x1 convs) -> l3 -> gate,
            # j-major with per-chunk yl2 tiles so l3 pipelines behind l2
            for j in range(4):
                sl = bass.ts(j, 512)
                yl2j = []
                for cb in range(2):
                    pl2 = ps.tile([128, 8, 64], F32, tag="l2", bufs=3)
                    first = True
                    for ci in range(2):
                        for ty in range(3):
                            for tx in range(3):
                                t = ty * 3 + tx
                                _mm(
                                    nc, pl2,
                                    l2w_cb[cb][:, t, ci, :],
                                    pad_sb[:, ci, j * 8 + ty : j * 8 + ty + 8,
                                           tx : tx + 64],
                                    first, t == 8 and ci == 1,
                                )
                                first = False
                    y2t = tmp.tile([128, 512], MM_DT, tag=f"y2{cb}")
                    nc.vector.tensor_scalar(
                        y2t,
                        pl2.rearrange("p a b -> p (a b)"),
                        l2b_sb[:, cb : cb + 1], 0.0,
                        AluOpType.add, AluOpType.max,
                    )
                    yl2j.append(y2t)
                for cb in range(2):
                    pl3 = ps.tile([128, 512], F32, tag="l3", bufs=2)
                    _mm(nc, pl3, l3w_sb[:, 0, bass.ts(cb, 128)], yl2j[0],
                        True, False)
                    _mm(nc, pl3, l3w_sb[:, 1, bass.ts(cb, 128)], yl2j[1],
                        False, True)
                    y3 = tmp.tile([128, 512], F32, tag="y3")
                    nc.vector.tensor_scalar(
                        y3, pl3, l3b_sb[:, cb : cb + 1], 0.0,
                        AluOpType.add, AluOpType.max,
                    )
                    gt = tmp.tile([128, 512], F32, tag="g")
                    nc.scalar.activation(
                        gt, y3, ACT.Sigmoid, bias=yr_sb[:, cb : cb + 1]
                    )
                    nc.vector.tensor_tensor(
                        out_sb[:, cb, sl], gt, feat_sb[:, cb, sl], AluOpType.mult
                    )
                    nc.sync.dma_start(out=out_d[cb][:, sl], in_=out_sb[:, cb, sl])

    _split_multi_waits(nc)
    return nc


# ---------------------------------------------------------------- host side

_CACHE = {}


def _programs():
    if "a" not in _CACHE:
        _CACHE["a"] = _build_phase_a()
        _CACHE["b"] = _build_phase_b()
    return _CACHE["a"], _CACHE["b"]


def _fold_bn(w, bias, g, bb, m, v):
    s = g / np.sqrt(v + EPS)
    t = bb - s * m
    wf = w * (s[:, None] if w.ndim == 2 else s[:, None, None, None])
    return wf.astype(np.float32), (s * bias + t).astype(np.float32)


def _prep_weights(inp):
    def fold(wk, bk2, pre):
        return _fold_bn(inp[wk], inp[bk2], inp[pre + "_g"], inp[pre + "_b"],
                        inp[pre + "_m"], inp[pre + "_v"])

    d1w, d1b = fold("d1_w", "d1_b", "bn1")
    l1w, l1b = fold("l1_w", "l1_b", "lbn1")
    l2w, l2b = fold("l2_w", "l2_b", "lbn2")
    l3w, l3b = fold("l3_w", "l3_b", "lbn3")
    r1w, r1b = fold("r1_w", "r1_b", "rbn")

    def pm(w):  # [ci, m] -> [128, 2*m] (per-partition packed, chunk-major)
        return np.ascontiguousarray(
            w.reshape(2, 128, -1).transpose(1, 0, 2).reshape(128, -1)
        )

    wv_aug = np.concatenate([inp["wv"].T, np.zeros((C, 2), np.float32)], axis=1)
    f32v = lambda a: np.ascontiguousarray(a).astype(np.float32)
    p = {}
    p["wpk"] = np.concatenate(
        [
            pm(inp["wq"].T), pm(inp["wk"].T), pm(wv_aug), pm(d1w.T), pm(l1w.T),
            np.eye(128, dtype=np.float32),
            f32v(d1b.reshape(2, 128).T), f32v(l1b.reshape(2, 128).T),
        ],
        axis=1,
    )
    p["wrow"] = np.concatenate(
        [inp["bq"], inp["bk"],
         np.concatenate([inp["bv"], [1.0, 0.0]]).astype(np.float32)]
    )[None, :]
    # phase B packs
    w2 = l2w.reshape(2, 128, 2, 128, 3, 3)  # [cb, co, ci_hi, ci_lo, ty, tx]
    l2p = np.ascontiguousarray(w2.transpose(3, 4, 5, 2, 0, 1))  # [ci_lo,ty,tx,ci_hi,cb,co]
    l2cb = [
        np.ascontiguousarray(l2p[:, :, :, :, cb, :]).reshape(128, -1)
        for cb in range(2)
    ]
    p["l2w0"] = l2cb[0]
    p["wpb"] = np.concatenate(
        [
            l2cb[1], pm(l3w.T), pm(r1w.T),
            f32v(inp["r2_w"].T),
            f32v(l2b.reshape(2, 128).T), f32v(l3b.reshape(2, 128).T),
            f32v(r1b[:, None]), f32v(inp["r2_b"].reshape(2, 128).T),
        ],
        axis=1,
    )
    return {k: np.ascontiguousarray(v.astype(np.float32)) for k, v in p.items()}


def _run_spmd(nc, in_maps):
    try:
        return run_bass_kernel_spmd(nc, in_maps, core_ids=list(range(NCORES)))
    except Exception:
        # transient NRT device errors have been observed; retry once
        return run_bass_kernel_spmd(nc, in_maps, core_ids=list(range(NCORES)))


def kernel(**inputs):
    nca, ncb = _programs()
    p = _prep_weights(inputs)
    x = inputs["x"].astype(np.float32).reshape(B, 2, 128, N)

    in_maps_a = []
    for core in range(NCORES):
        b, s = divmod(core, 2)
        m = {"wpk": p["wpk"], "wrow": p["wrow"]}
        if s == 0:
            m["xb"] = x[b]
        else:
            # rotate so this core's query half occupies columns [0, NH);
            # attention is permutation-invariant over key positions, so K/V
            # built from the rotated order give identical results.
            m["xb"] = np.ascontiguousarray(
                np.concatenate([x[b][:, :, NH:], x[b][:, :, :NH]], axis=2)
            )
        in_maps_a.append(m)
    res_a = _run_spmd(nca, in_maps_a)
    feats = [r["feat"] for r in res_a.results]
    yl1s = [r["yl1"].reshape(2, 128, 32, 64) for r in res_a.results]
    ysums = [r["ysum"] for r in res_a.results]

    zrow = np.zeros((2, 128, 1, 64), np.float32)
    in_maps_b = []
    for core in range(NCORES):
        b, s = divmod(core, 2)
        m = {"l2w0": p["l2w0"], "wpb": p["wpb"]}
        m["feat"] = feats[core]
        own, other = yl1s[core], yl1s[2 * b + (1 - s)]
        if s == 0:
            ylh = np.concatenate([zrow, own, other[:, :, 0:1]], axis=2)
        else:
            ylh = np.concatenate([other[:, :, 31:32], own, zrow], axis=2)
        ylh = np.pad(ylh, ((0, 0), (0, 0), (0, 0), (1, 1)))
        m["yl1h"] = np.ascontiguousarray(ylh).reshape(2, 128, 34 * 66)
        m["yss"] = np.concatenate([ysums[2 * b], ysums[2 * b + 1]], axis=1)
        in_maps_b.append(m)
    res_b = _run_spmd(ncb, in_maps_b)

    out = np.empty((B, C, H, W), np.float32)
    for core in range(NCORES):
        b, s = divmod(core, 2)
        out[b, :, s * 32 : (s + 1) * 32, :] = (
            res_b.results[core]["out"].reshape(C, 32, 64)
        )
    return out
